# revision 1
# baseline (speedup 1.0000x reference)
"""Grouped-experts SwiGLU MoE kernel for Trainium2 (8 NeuronCores).

Problem: T=8192 tokens (pre-sorted into contiguous per-expert blocks of
sizes num_tokens_per_expert), D=1024, H=2816, E=8 experts.
out[t] = (silu(x@w1^T) * (x@w3^T)) @ w2^T  with the owning expert's weights;
tokens past sum(counts) produce zeros.

Sharding: 8-way tensor-parallel split of the hidden dim H, exact 352
per core.  Every core processes ALL valid tokens of ALL experts for its
h-slice and emits partial outputs (contraction over h is split); the
host sums the 8 partials.  Every core's instruction stream is identical
(true SPMD) -- perfectly load-balanced regardless of expert imbalance.

GEMMs run in bf16 (PE 1 cycle/row) with fp32 PSUM accumulation.

PE-work minimization (a matmul's MOVING dim is unquantized; only
stationary/partition dims pay ceil-to-128):
- GEMM1/3 use a MIXED per-expert formulation: q = c//128 full token
  tiles run stationary-x (x tile stationary, the exact 352 h-columns
  streamed: 22 cyc/token), the r = c%128 remainder runs the 3-h-slot
  form (tokens streamed: 24r, exact).  2816q + 24r beats both pure
  forms.  Stationary-x output lands [token, h] and is DMA-transposed
  (XBAR, SP queue, off the PE) into the standard [h, token] h2 layout.
- The third h-slot covers the overlapping range h[224:352] (128 rows;
  the 32 overlap rows are zero-weighted in every expert's w2 k2 tile),
  which keeps h2's k2-plane row meaning consistent between transpose-
  and slot-written columns at zero PE cost.
- GEMM2 in [d, token] form: streams exactly count_e columns per expert;
  g d-tiles share one PSUM bank so one DVE copy drains them all.

Scheduling (PE ~95% busy, zero mid-kernel gaps):
- Software pipelining: GEMM2(prev expert) is emitted after GEMM1/3(curr
  expert), so the PE never waits on the silu/mul chain.
- x/out DRAM layouts put the d-block dim innermost -> every DMA is 128
  descriptors of large contiguous per-partition segments.
- DMA issue spread across the three DMA-capable queues in consumption
  order: x + w2 on SP, w1/w3 on Pool (SWDGE), small-expert stores on Act
  (so they never block the next round's weight loads), final store on SP.
- Expert order: descending count for prefetch pipelining, but led by the
  smallest >=256-token expert (pure slot form, tiny 64-col first chunk,
  h-tile-major duplicate weight tensors w1e/w3e) so the first matmul
  starts at the ~2.4us DMA-latency floor.
"""

import sys

sys.path.insert(0, "/opt/trn_rl_repo")

import numpy as np
import ml_dtypes

T, D, E = 8192, 1024, 8
H = 2816
CAP = T // E
NCORES = 8
HT = 3  # h-slots per core, ranges (0,128),(128,256),(224,352)
HSLICE = H // NCORES  # 352 exact, no padding
BF16 = ml_dtypes.bfloat16

_COMPILE_CACHE = {}
LAST_RESULTS = None  # BassKernelResults of the most recent device run


def _derive_cfg(counts):
    """Static structure derived from the per-expert token counts.
    Tokens are packed exactly (no padding): expert e owns packed columns
    [offs[e], offs[e]+counts[e]).  All GEMMs consume near-equal chunks of
    <=512 columns."""
    counts = [int(c) for c in counts]
    offs = [0]
    for c in counts:
        offs.append(offs[-1] + c)
    total_cols = offs[-1]
    # expert processing order: descending count, but lead with the smallest
    # expert >=256 (its x loads fast -> PE starts early; its compute covers
    # the big first expert's loads)
    desc = sorted([e for e in range(E) if counts[e] > 0],
                  key=lambda e: -counts[e])
    cands = [e for e in desc if counts[e] >= 256]
    if cands:
        e0 = min(cands, key=lambda e: counts[e])
        order = [e0] + [e for e in desc if e != e0]
    else:
        order = desc

    def _mkchunks(e, maxw):
        c = counts[e]
        out = []
        n = -(-c // maxw)
        base, rem = divmod(c, n)
        c0 = 0
        for i in range(n):
            w = base + (1 if i < rem else 0)
            out.append((offs[e] + c0, w))
            c0 += w
        return out

    # mixed GEMM1/3 formulation: q full 128-token tiles via stationary-x
    # (x stationary, 352 h-columns streamed exactly: 22 cyc/token) plus
    # the <128-token remainder via the 3-h-slot stream (24 cyc/token,
    # exact).  2816q + 24r beats both pure forms.  First expert stays
    # pure slot-form (its startup pacing needs h-tile-granular loads).
    mixed = {}  # e -> (q_tiles, remainder)
    for e in range(E):
        c = counts[e]
        if c == 0 or (order and e == order[0]):
            continue
        q, r = divmod(c, 128)
        if r >= 118:
            q, r = q + 1, 0  # partial tile still beats 24r here
        if q > 0:
            mixed[e] = (q, r)

    chunks = {}  # e -> [(col0, width<=512)]
    for e in range(E):
        if counts[e] == 0:
            chunks[e] = []
            continue
        # first expert: small chunks so the PE can start on the first
        # piece of x while the rest streams in (tiny first chunk)
        if order and e == order[0]:
            c = counts[e]
            if c > 64:
                first = [(offs[e], 64)]
                rest = []
                rem, c0 = c - 64, 64
                n = -(-rem // 128)
                base, extra = divmod(rem, n)
                for i in range(n):
                    w = base + (1 if i < extra else 0)
                    rest.append((offs[e] + c0, w))
                    c0 += w
                chunks[e] = first + rest
            else:
                chunks[e] = _mkchunks(e, 128)
        else:
            chunks[e] = _mkchunks(e, 512)
    return {
        "counts": counts,
        "offs": offs[:E],
        "total_cols": total_cols,
        "chunks": chunks,
        "order": order,
        "mixed": mixed,
    }


def _build_program(cfg, repeat=1):
    import concourse.bass as bass
    import concourse.bacc as bacc
    import concourse.mybir as mybir
    import concourse.tile as tile

    dt = mybir.dt
    COLS = cfg["total_cols"]
    counts = cfg["counts"]
    offs = cfg["offs"]
    chunks = cfg["chunks"]
    order = cfg["order"]

    nc = bacc.Bacc("TRN2", target_bir_lowering=False, debug=False,
                   num_devices=NCORES)

    # x packed columns: [p=128, col, do=8] with d = do*128 + p; the col-mid
    # layout keeps every x DMA at 128 descriptors (one 16B*w contiguous
    # segment per partition)
    xts = nc.dram_tensor("xts", [128, COLS, 8], dt.bfloat16,
                         kind="ExternalInput").ap()
    # weights pre-permuted on host; every DMA slice is contiguous per
    # partition row:
    # w1s/w3s: (E, p=128, do=8, h=352)  [d = do*128+p contracted; h exact]
    # w2s:     (E, p=128, kt=3, d=D)    [h = kt*128+p contracted;
    #                                    kt=2 rows 96:128 are zero pad]
    w1s = nc.dram_tensor("w1s", [E, 128, 8, HSLICE], dt.bfloat16,
                         kind="ExternalInput").ap()
    w3s = nc.dram_tensor("w3s", [E, 128, 8, HSLICE], dt.bfloat16,
                         kind="ExternalInput").ap()
    w2s = nc.dram_tensor("w2s", [E, 128, HT, D], dt.bfloat16,
                         kind="ExternalInput").ap()
    # first expert's w1/w3 duplicated in h-tile-major layout
    # (p, ht, do, hi): startup loads slice per h-tile with 2KB elements
    w1e = nc.dram_tensor("w1e", [128, HT, 8, 128], dt.bfloat16,
                         kind="ExternalInput").ap()
    w3e = nc.dram_tensor("w3e", [128, HT, 8, 128], dt.bfloat16,
                         kind="ExternalInput").ap()
    # partial output, [p=128, col, dt=8]: out[col, dt*128+p]
    outp = nc.dram_tensor("outp", [128, COLS, 8], dt.bfloat16,
                          kind="ExternalOutput").ap()

    with tile.TileContext(nc) as tc:
        with (
            tc.tile_pool(name="xpool", bufs=2) as xpool,
            tc.tile_pool(name="w1pool", bufs=2) as w1pool,
            tc.tile_pool(name="w3pool", bufs=2) as w3pool,
            tc.tile_pool(name="w2pool", bufs=3) as w2pool,
            tc.tile_pool(name="h2pool", bufs=3) as h2pool,
            tc.tile_pool(name="h2xpool", bufs=2) as h2xpool,
            tc.tile_pool(name="e0pool", bufs=1) as e0pool,
            tc.tile_pool(name="sgpool", bufs=3) as sgpool,
            tc.tile_pool(name="obpool", bufs=2) as obpool,
            tc.tile_pool(name="psgu", bufs=2, space="PSUM") as psgu,
            tc.tile_pool(name="pso", bufs=4, space="PSUM") as pso,
        ):
          for _rep in range(repeat):
            state = {}  # e -> (xe, w1t, w3t, w2t, h2)


            def emit_loads(e, pending_w2, first=False):
                xe = xpool.tile([128, 1024, 8], dt.bfloat16, tag="xe")
                w1t = w1pool.tile([128, 8, HSLICE], dt.bfloat16, tag="w1t")
                w3t = w3pool.tile([128, 8, HSLICE], dt.bfloat16, tag="w3t")
                w2t = w2pool.tile([128, HT, D], dt.bfloat16, tag="w2t")
                # x chunks + w2 on the SP queue; w1/w3 on the Pool (SWDGE)
                # queue: the queues transfer in parallel.  Act does only
                # silu (DMAs there would head-of-line-block the silu chain).
                if first:
                    # kernel startup: h-tile-major duplicate tensors, first
                    # d-tile of w1[h0] alone so the very first matmul's
                    # weights+x land ASAP
                    w1t = e0pool.tile([128, HT, 8, 128], dt.bfloat16,
                                      tag="w1et")
                    w3t = e0pool.tile([128, HT, 8, 128], dt.bfloat16,
                                      tag="w3et")
                    # first w1 d-tile via SP and first x chunk via Act:
                    # both ride the lower-latency HWDGE queues in parallel
                    nc.sync.dma_start(w1t[:, 0, 0:1], w1e[:, 0, 0:1])
                    nc.gpsimd.dma_start(w1t[:, 0, 1:], w1e[:, 0, 1:])
                    for ci, (col0, w) in enumerate(chunks[e]):
                        rel0 = col0 - offs[e]
                        eng = nc.scalar if ci == 0 else nc.sync
                        eng.dma_start(xe[:, rel0:rel0 + w, :],
                                      xts[:, col0:col0 + w, :])
                    nc.gpsimd.dma_start(w3t[:, 0], w3e[:, 0])
                    for h in range(1, HT):
                        nc.gpsimd.dma_start(w1t[:, h], w1e[:, h])
                        nc.gpsimd.dma_start(w3t[:, h], w3e[:, h])
                else:
                    nc.gpsimd.dma_start(w1t[:, :, :], w1s[e][:, :, :])
                    for (col0, w) in chunks[e]:
                        rel0 = col0 - offs[e]
                        nc.sync.dma_start(xe[:, rel0:rel0 + w, :],
                                          xts[:, col0:col0 + w, :])
                    nc.gpsimd.dma_start(w3t[:, :, :], w3s[e][:, :, :])
                # w2 of the PREVIOUS expert after this expert's x (not
                # needed until its GEMM2, which runs an expert later)
                for (pe, pw2) in pending_w2:
                    nc.sync.dma_start(pw2[:, :, :], w2s[pe][:, :, :])
                state[e] = (xe, w1t, w3t, w2t, None, first)
                return w2t

            def emit_g13(e):
                xe, w1t, w3t, w2t, _, efirst = state[e]
                h2 = h2pool.tile([128, HT, 1024], dt.bfloat16, tag="h2")
                c = counts[e]
                q, r = cfg["mixed"].get(e, (0, c))
                if q:
                    # stationary-x tiles: x token-tiles stationary, 352
                    # h-columns stream exactly; DMA-transpose into the
                    # standard [h, token] h2 layout off the PE
                    h2x = h2xpool.tile([128, 8, HSLICE], dt.bfloat16,
                                       tag="h2x")
                    for tt in range(q):
                        t0 = tt * 128
                        m = min(128, c - t0)
                        pg = psgu.tile([128, 512], dt.float32, tag="pg")
                        pu = psgu.tile([128, 512], dt.float32, tag="pu")
                        for d in range(8):
                            nc.tensor.matmul(
                                pg[:m, :HSLICE],
                                xe[:, t0:t0 + m, d], w1t[:, d, :],
                                start=(d == 0), stop=(d == 7))
                        for d in range(8):
                            nc.tensor.matmul(
                                pu[:m, :HSLICE],
                                xe[:, t0:t0 + m, d], w3t[:, d, :],
                                start=(d == 0), stop=(d == 7))
                        sg = sgpool.tile([128, 512], dt.float32, tag="sg")
                        nc.scalar.activation(
                            sg[:m, :HSLICE], pg[:m, :HSLICE],
                            mybir.ActivationFunctionType.Silu)
                        nc.vector.tensor_mul(
                            out=h2x[:m, tt % 8, :],
                            in0=sg[:m, :HSLICE], in1=pu[:m, :HSLICE])
                        # transpose blocks must be 128 cols x 16n rows;
                        # the k2 block reads overlapping h[224:352] -- its
                        # first 32 rows are zero-weighted in w2's k2 tile
                        mt = min(128, -(-m // 16) * 16)
                        for k, hc0 in enumerate((0, 128, 224)):
                            nc.sync.dma_start(
                                h2[:, k, t0:t0 + mt],
                                h2x[:mt, tt % 8, hc0:hc0 + 128],
                                transpose=True)
                # remainder (or whole expert) in slot form: h-slots
                # (0,128),(128,256),(224,352) -- slot 3 overlaps, its
                # first 32 rows zero-weighted in w2; streams are exact
                swchunks = ([(offs[e] + q * 128, r)] if e in cfg["mixed"]
                            else chunks[e]) if r else []
                for h, hc0 in enumerate((0, 128, 224)):
                    for (col0, w) in swchunks:
                        rel0 = col0 - offs[e]
                        pg = psgu.tile([128, 512], dt.float32, tag="pg")
                        pu = psgu.tile([128, 512], dt.float32, tag="pu")
                        for d in range(8):
                            lhs = (w1t[:, h, d, :] if efirst else
                                   w1t[:, d, hc0:hc0 + 128])
                            nc.tensor.matmul(
                                pg[:, :w], lhs,
                                xe[:, rel0:rel0 + w, d],
                                start=(d == 0), stop=(d == 7))
                        for d in range(8):
                            lhs = (w3t[:, h, d, :] if efirst else
                                   w3t[:, d, hc0:hc0 + 128])
                            nc.tensor.matmul(
                                pu[:, :w], lhs,
                                xe[:, rel0:rel0 + w, d],
                                start=(d == 0), stop=(d == 7))
                        sg = sgpool.tile([128, 512], dt.float32, tag="sg")
                        nc.scalar.activation(
                            sg[:, :w], pg[:, :w],
                            mybir.ActivationFunctionType.Silu)
                        nc.vector.tensor_mul(
                            out=h2[:, h, rel0:rel0 + w],
                            in0=sg[:, :w], in1=pu[:, :w])
                state[e] = (xe, w1t, w3t, w2t, h2, efirst)

            def emit_g2(e, last=False):
                _, _, _, w2t, h2, _ = state[e]
                for (col0, w) in chunks[e]:
                    rel0 = col0 - offs[e]
                    # pack g d-tiles into one PSUM bank -> one copy per
                    # group (amortizes the per-copy fixed cost); the very
                    # last chunk caps g at 4 so its final copy overlaps PE
                    g = 8 if w <= 64 else 4 if w <= 128 else 2 if w <= 256 else 1
                    if last and (col0, w) == chunks[e][-1]:
                        g = min(g, 4)
                    ob = obpool.tile([128, 512, 8], dt.bfloat16, tag="ob")
                    for dt0 in range(0, 8, g):
                        po = pso.tile([128, 512], dt.float32, tag="po")
                        for gi in range(g):
                            dti = dt0 + gi
                            for k in range(HT):
                                nc.tensor.matmul(
                                    po[:, gi * w:gi * w + w],
                                    w2t[:, k, dti * 128:(dti + 1) * 128],
                                    h2[:, k, rel0:rel0 + w],
                                    start=(k == 0), stop=(k == HT - 1))
                        # ob view (dt, col): permute so it matches po's
                        # flat (dt-major) layout
                        obv = ob[:, :w, dt0:dt0 + g].transpose([0, 2, 1])
                        nc.vector.tensor_copy(out=obv, in_=po[:, :g * w])
                    # final store on SP (lower completion latency in the
                    # end-of-kernel drain); small-expert stores on Act so
                    # they don't head-of-line-block the Pool queue's weight
                    # loads at the repeat boundary; the rest on Pool
                    if last:
                        eng = nc.sync
                    elif counts[e] < 320:
                        eng = nc.scalar
                    else:
                        eng = nc.gpsimd
                    eng.dma_start(outp[:, col0:col0 + w, :],
                                  ob[:, :w, :])

            pending_w2 = []
            for i, e in enumerate(order):
                w2t = emit_loads(e, pending_w2, first=(i == 0))
                pending_w2 = [(e, w2t)]
                emit_g13(e)
                if i >= 1:
                    emit_g2(order[i - 1])
            if order:
                # last expert's w2 load was deferred; issue it now
                for (pe, pw2) in pending_w2:
                    nc.sync.dma_start(pw2[:, :, :], w2s[pe][:, :, :])
                emit_g2(order[-1], last=True)

    nc.compile()
    return nc


def _get_program(cfg, repeat=1):
    key = (tuple(cfg["counts"]), repeat)
    if key not in _COMPILE_CACHE:
        _COMPILE_CACHE[key] = _build_program(cfg, repeat)
    return _COMPILE_CACHE[key]


def _pack_inputs(x, counts, w1, w2, w3, cfg):
    """Build per-core input maps (host-side routing + layout)."""
    offs, COLS = cfg["offs"], cfg["total_cols"]

    # packed x: all valid tokens, exactly packed per expert
    xpack = np.zeros((COLS, D), np.float32)
    starts = np.concatenate([[0], np.cumsum(counts)]).astype(np.int64)
    for e in range(E):
        c = int(counts[e])
        if c:
            xpack[offs[e]:offs[e] + c] = x[starts[e]:starts[e] + c]
    # (COLS, D) -> (col, do=8, p=128) -> (p, col, do)
    xts = np.ascontiguousarray(
        xpack.astype(BF16).reshape(COLS, 8, 128).transpose(2, 0, 1))

    # weights: transpose so the contraction dim leads; exact H split
    # (HSLICE=352/core); w2 k-tiles padded to 128 rows with zeros
    w1b = w1.astype(BF16)
    w3b = w3.astype(BF16)
    w2b = w2.astype(BF16)
    # (E, D, H)
    w1T = np.ascontiguousarray(np.transpose(w1b, (0, 2, 1)))
    w3T = np.ascontiguousarray(np.transpose(w3b, (0, 2, 1)))
    # (E, H, D)
    w2T = np.ascontiguousarray(np.transpose(w2b, (0, 2, 1)))

    in_maps = []
    for c in range(NCORES):
        hs = slice(c * HSLICE, (c + 1) * HSLICE)
        # w1/w3: (D, HSLICE) -> (do, p, h) -> (p, do, h)
        w1c = w1T[:, :, hs].reshape(E, 8, 128, HSLICE).transpose(0, 2, 1, 3)
        w3c = w3T[:, :, hs].reshape(E, 8, 128, HSLICE).transpose(0, 2, 1, 3)
        # w2: (HSLICE, D) -> 3 k-tiles of 128 rows -> (p, kt, d).
        # k2 = h[224:352] for every expert, with the first 32 rows zeroed
        # (that h range is counted in k1; the overlap matches the h2
        # k2-plane produced by both the transposes and the slot form)
        w2p = np.zeros((E, HT, 128, D), BF16)
        w2p[:, 0] = w2T[:, hs, :][:, 0:128]
        w2p[:, 1] = w2T[:, hs, :][:, 128:256]
        w2p[:, 2, 32:128] = w2T[:, hs, :][:, 256:352]
        w2c = w2p.transpose(0, 2, 1, 3)
        # first expert's w1/w3 duplicated in h-tile-major layout with the
        # same overlapping slot-3 range
        e0 = cfg["order"][0] if cfg["order"] else 0
        w1ec = np.zeros((128, HT, 8, 128), BF16)
        w3ec = np.zeros((128, HT, 8, 128), BF16)
        for ht, hc0 in enumerate((0, 128, 224)):
            w1ec[:, ht, :, :] = w1c[e0][:, :, hc0:hc0 + 128]
            w3ec[:, ht, :, :] = w3c[e0][:, :, hc0:hc0 + 128]
        in_maps.append({
            "xts": xts,
            "w1s": np.ascontiguousarray(w1c),
            "w3s": np.ascontiguousarray(w3c),
            "w2s": np.ascontiguousarray(w2c),
            "w1e": w1ec,
            "w3e": w3ec,
        })
    return in_maps, starts


def _unpack_output(results, counts, cfg, starts):
    offs = cfg["offs"]
    COLS = cfg["total_cols"]
    acc = np.zeros((COLS, D), np.float32)
    for r in results:
        # outp: (p, col, dt) with d = dt*128 + p -> (col, d)
        acc += r["outp"].astype(np.float32).transpose(1, 2, 0).reshape(COLS, D)
    out = np.zeros((T, D), np.float32)
    for e in range(E):
        c = int(counts[e])
        if c:
            out[starts[e]:starts[e] + c] = acc[offs[e]:offs[e] + c]
    return out


def kernel(x, num_tokens_per_expert, w1, w2, w3):
    global LAST_RESULTS
    counts = np.asarray(num_tokens_per_expert).astype(np.int64)
    cfg = _derive_cfg(counts)
    if cfg["total_cols"] == 0:
        return np.zeros((T, D), np.float32)

    nc = _get_program(cfg)
    in_maps, starts = _pack_inputs(
        np.asarray(x, np.float32), counts,
        np.asarray(w1, np.float32), np.asarray(w2, np.float32),
        np.asarray(w3, np.float32), cfg)

    from concourse.bass_utils import run_bass_kernel_spmd
    res = run_bass_kernel_spmd(nc, in_maps, list(range(NCORES)))
    LAST_RESULTS = res
    return _unpack_output(res.results, counts, cfg, starts)



# revision 2
# speedup vs baseline: 2.2539x; 2.2539x over previous
"""Grouped-experts SwiGLU MoE kernel for Trainium2 (8 NeuronCores).

Problem: T=8192 tokens (pre-sorted into contiguous per-expert blocks of
sizes num_tokens_per_expert), D=1024, H=2816, E=8 experts.
out[t] = (silu(x@w1^T) * (x@w3^T)) @ w2^T  with the owning expert's weights;
tokens past sum(counts) produce zeros.

Sharding: 8-way tensor-parallel split of the hidden dim H, exact 352
per core.  Every core processes ALL valid tokens of ALL experts for its
h-slice and emits partial outputs (contraction over h is split); the
host sums the 8 partials.  Every core's instruction stream is identical
(true SPMD) -- perfectly load-balanced regardless of expert imbalance.

GEMMs run in bf16 (PE 1 cycle/row) with fp32 PSUM accumulation.

All GEMMs stream packed token columns (moving dim = tokens, <=512 per
chunk).  GEMM1/3 use the 3-h-slot form: slots cover h ranges
(0,128),(128,256),(224,352); the third slot overlaps by 32 rows, which
are zero-weighted in every expert's w2 k2 tile, so all 128 partitions
of every h2 plane carry defined values at zero extra PE cost.  GEMM2
runs in [d, token] form: g d-tiles share one PSUM bank so one DVE copy
drains them all.

No DMA transposes anywhere: HW-measured transpose cost is ~1.2-3.6us
each (vs ~0.2us modeled), so the transpose-based stationary-x
formulation loses ~100us/rep of HWDGE ring time to save ~7us of PE.

Scheduling:
- Software pipelining: GEMM2(prev expert) is emitted after GEMM1/3(curr
  expert), so the PE never waits on the silu/mul chain; w2(e) loads are
  deferred by one expert as well.
- x/out DRAM layouts put the d-block dim innermost -> every DMA is 128
  descriptors of large contiguous per-partition segments.
- DMA spread: x + w2 on SP (HWDGE), w1/w3 on Pool (SWDGE), stores on
  Act except the final store on SP (lower completion latency in the
  end-of-kernel drain).
"""

import sys

sys.path.insert(0, "/opt/trn_rl_repo")

import numpy as np
import ml_dtypes

T, D, E = 8192, 1024, 8
H = 2816
CAP = T // E
NCORES = 8
HT = 3  # h-slots per core, ranges (0,128),(128,256),(224,352)
HSLICE = H // NCORES  # 352 exact, no padding
BF16 = ml_dtypes.bfloat16

_COMPILE_CACHE = {}
LAST_RESULTS = None  # BassKernelResults of the most recent device run


def _derive_cfg(counts):
    """Static structure derived from the per-expert token counts.
    Tokens are packed exactly (no padding): expert e owns packed columns
    [offs[e], offs[e]+counts[e]).  All GEMMs consume near-equal chunks of
    <=512 columns."""
    counts = [int(c) for c in counts]
    offs = [0]
    for c in counts:
        offs.append(offs[-1] + c)
    total_cols = offs[-1]
    order = sorted([e for e in range(E) if counts[e] > 0],
                   key=lambda e: -counts[e])

    def _mkchunks(e, maxw):
        c = counts[e]
        out = []
        n = -(-c // maxw)
        base, rem = divmod(c, n)
        c0 = 0
        for i in range(n):
            w = base + (1 if i < rem else 0)
            out.append((offs[e] + c0, w))
            c0 += w
        return out

    chunks = {e: (_mkchunks(e, 512) if counts[e] else []) for e in range(E)}
    return {
        "counts": counts,
        "offs": offs[:E],
        "total_cols": total_cols,
        "chunks": chunks,
        "order": order,
    }


def _build_program(cfg, repeat=1, hw_loop=False):
    import concourse.bass as bass
    import concourse.bacc as bacc
    import concourse.mybir as mybir
    import concourse.tile as tile

    dt = mybir.dt
    COLS = cfg["total_cols"]
    counts = cfg["counts"]
    offs = cfg["offs"]
    chunks = cfg["chunks"]
    order = cfg["order"]

    nc = bacc.Bacc("TRN2", target_bir_lowering=False, debug=False,
                   num_devices=NCORES)

    # x packed columns: [p=128, do=8, col] with d = do*128 + p; cols
    # innermost so the slot-form MOVING operand xe[:, d, c0:c0+w] is a
    # contiguous 2B-stride stream (strided moving reads run ~4x slower
    # on HW); each x DMA is 128x8 descriptors of 2w-byte segments
    xts = nc.dram_tensor("xts", [128, 8, COLS], dt.bfloat16,
                         kind="ExternalInput").ap()
    # weights pre-permuted on host; every DMA slice is contiguous per
    # partition row:
    # w1s/w3s: (E, p=128, do=8, h=352)  [d = do*128+p contracted; h exact]
    # w2s:     (E, p=128, kt=3, d=D)    [h = kt*128+p contracted;
    #                                    kt=2 rows 0:32 are zero pad]
    w1s = nc.dram_tensor("w1s", [E, 128, 8, HSLICE], dt.bfloat16,
                         kind="ExternalInput").ap()
    w3s = nc.dram_tensor("w3s", [E, 128, 8, HSLICE], dt.bfloat16,
                         kind="ExternalInput").ap()
    w2s = nc.dram_tensor("w2s", [E, 128, HT, D], dt.bfloat16,
                         kind="ExternalInput").ap()
    # partial output, [p=128, col, dt=8]: out[col, dt*128+p]
    outp = nc.dram_tensor("outp", [128, COLS, 8], dt.bfloat16,
                          kind="ExternalOutput").ap()

    with tile.TileContext(nc) as tc:
        with (
            tc.tile_pool(name="xpool", bufs=2) as xpool,
            tc.tile_pool(name="w1pool", bufs=2) as w1pool,
            tc.tile_pool(name="w3pool", bufs=2) as w3pool,
            tc.tile_pool(name="w2pool", bufs=3) as w2pool,
            tc.tile_pool(name="h2pool", bufs=3) as h2pool,
            tc.tile_pool(name="sgpool", bufs=3) as sgpool,
            tc.tile_pool(name="obpool", bufs=2) as obpool,
            tc.tile_pool(name="psgu", bufs=2, space="PSUM") as psgu,
            tc.tile_pool(name="pso", bufs=4, space="PSUM") as pso,
        ):
          def _body():
            state = {}  # e -> (xe, w1t, w3t, w2t, h2)

            def emit_loads(e, pending_w2):
                xe = xpool.tile([128, 8, 1024], dt.bfloat16, tag="xe")
                w1t = w1pool.tile([128, 8, HSLICE], dt.bfloat16, tag="w1t")
                w3t = w3pool.tile([128, 8, HSLICE], dt.bfloat16, tag="w3t")
                w2t = w2pool.tile([128, HT, D], dt.bfloat16, tag="w2t")
                # x chunks + w2 on the SP queue; w1/w3 on the Pool (SWDGE)
                # queue: the queues transfer in parallel.
                nc.gpsimd.dma_start(w1t[:, :, :], w1s[e][:, :, :])
                for (col0, w) in chunks[e]:
                    rel0 = col0 - offs[e]
                    nc.sync.dma_start(xe[:, :, rel0:rel0 + w],
                                      xts[:, :, col0:col0 + w])
                nc.gpsimd.dma_start(w3t[:, :, :], w3s[e][:, :, :])
                # w2 of the PREVIOUS expert after this expert's x (not
                # needed until its GEMM2, which runs an expert later)
                for (pe, pw2) in pending_w2:
                    nc.sync.dma_start(pw2[:, :, :], w2s[pe][:, :, :])
                state[e] = (xe, w1t, w3t, w2t, None)
                return w2t

            def emit_g13(e):
                xe, w1t, w3t, w2t, _ = state[e]
                h2 = h2pool.tile([128, HT, 1024], dt.bfloat16, tag="h2")
                # 3-h-slot form: slots (0,128),(128,256),(224,352); slot 3
                # overlaps by 32 rows, zero-weighted in w2's k2 tile
                for h, hc0 in enumerate((0, 128, 224)):
                    for (col0, w) in chunks[e]:
                        rel0 = col0 - offs[e]
                        pg = psgu.tile([128, 512], dt.float32, tag="pg")
                        pu = psgu.tile([128, 512], dt.float32, tag="pu")
                        for d in range(8):
                            nc.tensor.matmul(
                                pg[:, :w], w1t[:, d, hc0:hc0 + 128],
                                xe[:, d, rel0:rel0 + w],
                                start=(d == 0), stop=(d == 7))
                        for d in range(8):
                            nc.tensor.matmul(
                                pu[:, :w], w3t[:, d, hc0:hc0 + 128],
                                xe[:, d, rel0:rel0 + w],
                                start=(d == 0), stop=(d == 7))
                        sg = sgpool.tile([128, 512], dt.float32, tag="sg")
                        nc.scalar.activation(
                            sg[:, :w], pg[:, :w],
                            mybir.ActivationFunctionType.Silu)
                        nc.vector.tensor_mul(
                            out=h2[:, h, rel0:rel0 + w],
                            in0=sg[:, :w], in1=pu[:, :w])
                state[e] = (xe, w1t, w3t, w2t, h2)

            def emit_g2(e, last=False):
                _, _, _, w2t, h2 = state[e]
                for (col0, w) in chunks[e]:
                    rel0 = col0 - offs[e]
                    # pack g d-tiles into one PSUM bank -> one copy per
                    # group (amortizes the per-copy fixed cost); the very
                    # last chunk caps g at 4 so its final copy overlaps PE
                    g = 8 if w <= 64 else 4 if w <= 128 else 2 if w <= 256 else 1
                    if last and (col0, w) == chunks[e][-1]:
                        g = min(g, 4)
                    ob = obpool.tile([128, 512, 8], dt.bfloat16, tag="ob")
                    for dt0 in range(0, 8, g):
                        po = pso.tile([128, 512], dt.float32, tag="po")
                        for gi in range(g):
                            dti = dt0 + gi
                            for k in range(HT):
                                nc.tensor.matmul(
                                    po[:, gi * w:gi * w + w],
                                    w2t[:, k, dti * 128:(dti + 1) * 128],
                                    h2[:, k, rel0:rel0 + w],
                                    start=(k == 0), stop=(k == HT - 1))
                        # ob view (dt, col): permute so it matches po's
                        # flat (dt-major) layout
                        obv = ob[:, :w, dt0:dt0 + g].transpose([0, 2, 1])
                        nc.vector.tensor_copy(out=obv, in_=po[:, :g * w])
                    if last:
                        eng = nc.sync
                    elif counts[e] < 320:
                        eng = nc.scalar
                    else:
                        eng = nc.gpsimd
                    eng.dma_start(outp[:, col0:col0 + w, :],
                                  ob[:, :w, :])

            pending_w2 = []
            for i, e in enumerate(order):
                w2t = emit_loads(e, pending_w2)
                pending_w2 = [(e, w2t)]
                emit_g13(e)
                if i >= 1:
                    emit_g2(order[i - 1])
            if order:
                # last expert's w2 load was deferred; issue it now
                for (pe, pw2) in pending_w2:
                    nc.sync.dma_start(pw2[:, :, :], w2s[pe][:, :, :])
                emit_g2(order[-1], last=True)

          if hw_loop and repeat > 1:
            with tc.For_i(0, repeat, 1):
                _body()
          else:
            for _rep in range(repeat):
                _body()

    nc.compile()
    return nc


def _get_program(cfg, repeat=1, hw_loop=False):
    key = (tuple(cfg["counts"]), repeat, hw_loop)
    if key not in _COMPILE_CACHE:
        _COMPILE_CACHE[key] = _build_program(cfg, repeat, hw_loop)
    return _COMPILE_CACHE[key]


def _pack_inputs(x, counts, w1, w2, w3, cfg):
    """Build per-core input maps (host-side routing + layout)."""
    offs, COLS = cfg["offs"], cfg["total_cols"]

    # packed x: all valid tokens, exactly packed per expert
    xpack = np.zeros((COLS, D), np.float32)
    starts = np.concatenate([[0], np.cumsum(counts)]).astype(np.int64)
    for e in range(E):
        c = int(counts[e])
        if c:
            xpack[offs[e]:offs[e] + c] = x[starts[e]:starts[e] + c]
    # (COLS, D) -> (col, do=8, p=128) -> (p, do, col)
    xts = np.ascontiguousarray(
        xpack.astype(BF16).reshape(COLS, 8, 128).transpose(2, 1, 0))

    # weights: transpose so the contraction dim leads; exact H split
    # (HSLICE=352/core); w2 k-tiles padded to 128 rows with zeros
    w1b = w1.astype(BF16)
    w3b = w3.astype(BF16)
    w2b = w2.astype(BF16)
    # (E, D, H)
    w1T = np.ascontiguousarray(np.transpose(w1b, (0, 2, 1)))
    w3T = np.ascontiguousarray(np.transpose(w3b, (0, 2, 1)))
    # (E, H, D)
    w2T = np.ascontiguousarray(np.transpose(w2b, (0, 2, 1)))

    in_maps = []
    for c in range(NCORES):
        hs = slice(c * HSLICE, (c + 1) * HSLICE)
        # w1/w3: (D, HSLICE) -> (do, p, h) -> (p, do, h)
        w1c = w1T[:, :, hs].reshape(E, 8, 128, HSLICE).transpose(0, 2, 1, 3)
        w3c = w3T[:, :, hs].reshape(E, 8, 128, HSLICE).transpose(0, 2, 1, 3)
        # w2: (HSLICE, D) -> 3 k-tiles of 128 rows -> (p, kt, d).
        # k2 = h[224:352] for every expert, with the first 32 rows zeroed
        # (that h range is counted in k1; the overlap matches the h2
        # k2-plane produced by the slot form)
        w2p = np.zeros((E, HT, 128, D), BF16)
        w2p[:, 0] = w2T[:, hs, :][:, 0:128]
        w2p[:, 1] = w2T[:, hs, :][:, 128:256]
        w2p[:, 2, 32:128] = w2T[:, hs, :][:, 256:352]
        w2c = w2p.transpose(0, 2, 1, 3)
        in_maps.append({
            "xts": xts,
            "w1s": np.ascontiguousarray(w1c),
            "w3s": np.ascontiguousarray(w3c),
            "w2s": np.ascontiguousarray(w2c),
        })
    return in_maps, starts


def _unpack_output(results, counts, cfg, starts):
    offs = cfg["offs"]
    COLS = cfg["total_cols"]
    acc = np.zeros((COLS, D), np.float32)
    for r in results:
        # outp: (p, col, dt) with d = dt*128 + p -> (col, d)
        acc += r["outp"].astype(np.float32).transpose(1, 2, 0).reshape(COLS, D)
    out = np.zeros((T, D), np.float32)
    for e in range(E):
        c = int(counts[e])
        if c:
            out[starts[e]:starts[e] + c] = acc[offs[e]:offs[e] + c]
    return out


def kernel(x, num_tokens_per_expert, w1, w2, w3):
    global LAST_RESULTS
    counts = np.asarray(num_tokens_per_expert).astype(np.int64)
    cfg = _derive_cfg(counts)
    if cfg["total_cols"] == 0:
        return np.zeros((T, D), np.float32)

    nc = _get_program(cfg)
    in_maps, starts = _pack_inputs(
        np.asarray(x, np.float32), counts,
        np.asarray(w1, np.float32), np.asarray(w2, np.float32),
        np.asarray(w3, np.float32), cfg)

    from concourse.bass_utils import run_bass_kernel_spmd
    res = run_bass_kernel_spmd(nc, in_maps, list(range(NCORES)))
    LAST_RESULTS = res
    return _unpack_output(res.results, counts, cfg, starts)
